# revision 8
# baseline (speedup 1.0000x reference)
"""Trainium2 Bass kernel for the soft-LUT cellular-ASIC module.

Math per layer:  state'[b,hw] = clip( sum_p tw[l,p,hw] * prod_m f(c_m, bit_m(p)) )
where c_m[b,hw] = state[b,(h+i)%32,(w-1+j)%32]  (m = i*3+j),  f(c,0)=1-c, f(c,1)=c,
tw = sigmoid(toggle_gates).  bit_m(p) = bit (8-m) of p, i.e. m=0 is the MSB.

Implementation: sum-factorization ("soft-LUT tree").  Per 128-position tile,
A0 = sigmoid(tgT) laid out [128 positions, 512 combos]; 9 lerp steps
A'[q] = A[q] + c_m * (A[q+S] - A[q]) halve the free dim 512 -> 1.  LUT weights
sum to 1 (convex), so clip is a numerical no-op (kept as one cheap op).

Position layout ("PM"): tile t = b*8+th holds 128 positions p = ph*32+w with
h = th*4+ph; so hw = th*128 + p and state lives in SBUF as [128, 16].
Window gathers go through a DRAM array G[b, phx(6), wc(34), th(8)] --
G[b,phx,wc,th] = state[b, (th*4+phx)%32, (wc-1)%32] -- built with duplicated
row-phase (phx 4..5) and wrapped columns so that both the G-build DMAs (from
the [128,16] state tile) and the 9 window gathers G[b, i:i+4, j:j+32, :] are
affine <=3-dim DMAs with contiguous inner dims.

Sharding: data-parallel over batch B=16 across 8 cores (B_local=2, no comms).
"""

import numpy as np

import concourse.bass as bass
import concourse.bacc as bacc
import concourse.mybir as mybir
from concourse import tile
from concourse.bass_utils import run_bass_kernel_spmd

F32 = mybir.dt.float32
AF = mybir.ActivationFunctionType
OP = mybir.AluOpType

L = 4          # layers
NPOS = 512     # 2^9 LUT combos
HW = 1024      # 32*32 grid
BLOC = 2       # batch per core (16 / 8 cores)
NT = 16        # position tiles of 128 (BLOC*HW/128)
NCORES = 8

_CACHE = {}


def _emit_g_build(nc, G, sp, b):
    """Write PM-halo G[b] from state tile sp [128,16] (tiles b*8..b*8+7)."""
    bb = b * 8
    # main: phx 0..3, wc 1..32  <- state rows th*4+phx, col wc-1
    nc.sync.dma_start(out=G[b, 0:4, 1:33, :], in_=sp[:, bb : bb + 8])
    # phx 4..5 = rows (th+1)*4 + e, th 0..6
    nc.sync.dma_start(out=G[b, 4:6, 1:33, 0:7], in_=sp[0:64, bb + 1 : bb + 8])
    # phx 4..5, th=7: rows 32,33 wrap to rows 0,1
    nc.sync.dma_start(out=G[b, 4:6, 1:33, 7:8], in_=sp[0:64, bb : bb + 1])
    # wc=0 <- col 31
    nc.sync.dma_start(out=G[b, 0:4, 0, :], in_=sp[31:128:32, bb : bb + 8])
    nc.sync.dma_start(out=G[b, 4:6, 0, 0:7], in_=sp[31:64:32, bb + 1 : bb + 8])
    nc.sync.dma_start(out=G[b, 4:6, 0, 7:8], in_=sp[31:64:32, bb : bb + 1])
    # wc=33 <- col 0
    nc.sync.dma_start(out=G[b, 0:4, 33, :], in_=sp[0:128:32, bb : bb + 8])
    nc.sync.dma_start(out=G[b, 4:6, 33, 0:7], in_=sp[0:64:32, bb + 1 : bb + 8])
    nc.sync.dma_start(out=G[b, 4:6, 33, 7:8], in_=sp[0:64:32, bb : bb + 1])


def _build():
    nc = bacc.Bacc("TRN2", target_bir_lowering=False, debug=True)

    g0 = nc.declare_dram_parameter("g0", [BLOC, 6, 34, 8], F32, isOutput=False)
    tgt = nc.declare_dram_parameter("tgt", [L, HW, NPOS], F32, isOutput=False)
    out = nc.declare_dram_parameter("out", [128, NT], F32, isOutput=True)

    with tile.TileContext(nc) as tc:
        with (
            tc.tile_pool(name="dram", bufs=2, space="DRAM") as dram,
            tc.tile_pool(name="tg", bufs=4) as tgp,
            tc.tile_pool(name="a0", bufs=10) as a0p,
            tc.tile_pool(name="conv", bufs=2) as convp,
            tc.tile_pool(name="st", bufs=2) as stp,
            tc.tile_pool(name="wk", bufs=4) as wk,
        ):
            state = None  # [128, 16] PM layout
            for l in range(L):
                if l == 0:
                    G = g0
                else:
                    G = dram.tile([BLOC, 6, 34, 8], F32)
                    for b in range(BLOC):
                        _emit_g_build(nc, G, state_d, b)

                # ---- gather conv scalars: conv[p, m*16 + b*8 + th] = c_m
                conv = convp.tile([128, 9 * NT], F32)
                for i in range(3):
                    for j in range(3):
                        m = i * 3 + j
                        for b in range(BLOC):
                            nc.sync.dma_start(
                                out=conv[:, m * NT + b * 8 : m * NT + b * 8 + 8],
                                in_=G[b, i : i + 4, j : j + 32, :],
                            )

                # ---- A0 = sigmoid(tgT) per hw-block (shared by both b)
                a0 = []
                for hb in range(8):
                    tgsb = tgp.tile([128, NPOS], F32)
                    nc.sync.dma_start(
                        out=tgsb[:, :], in_=tgt[l, hb * 128 : (hb + 1) * 128, :]
                    )
                    a0t = a0p.tile([128, NPOS], F32)
                    nc.scalar.activation(a0t[:, :], tgsb[:, :], AF.Sigmoid)
                    a0.append(a0t)

                # ---- contraction tree per position tile
                newstate = stp.tile([128, NT], F32)
                for t in range(NT):
                    cur = a0[t % 8][:, :]
                    for s in range(9):
                        S = 256 >> s
                        c = conv[:, s * NT + t : s * NT + t + 1]
                        d = wk.tile([128, S], F32, tag=f"d{S}")
                        nc.vector.tensor_sub(d[:, :], cur[:, S : 2 * S], cur[:, 0:S])
                        if s == 8:
                            nxt_ap = newstate[:, t : t + 1]
                        else:
                            nxt = wk.tile([128, S], F32, tag=f"a{S}")
                            nxt_ap = nxt[:, :]
                        nc.vector.scalar_tensor_tensor(
                            nxt_ap, d[:, :], c, cur[:, 0:S], OP.mult, OP.add
                        )
                        cur = nxt_ap
                # clip to [0,1] (convexity makes this a numerical no-op; cheap safety)
                nc.vector.tensor_scalar(
                    newstate[:, :], newstate[:, :], 0.0, 1.0, OP.max, OP.min
                )
                state = newstate
                if l < L - 1:
                    # state to DRAM; G is then built DRAM->DRAM (clean deps)
                    state_d = dram.tile([128, NT], F32, tag="state_d")
                    nc.sync.dma_start(out=state_d[:, :], in_=newstate[:, :])

            # ---- write out in PM layout; host unpermutes
            nc.sync.dma_start(out=out[:, :], in_=state[:, :])

    nc.finalize()
    return nc


def _host_g0(xb):
    """Build PM-halo G for the initial state (xb: [2,32,32])."""
    phx = np.arange(6)
    th = np.arange(8)
    hr = (4 * th[None, :] + phx[:, None]) % 32          # [6, 8]
    wc = (np.arange(34) - 1) % 32                       # [34]
    return np.ascontiguousarray(
        xb[:, hr[:, None, :], wc[None, :, None]], dtype=np.float32
    )  # [2, 6, 34, 8]


def _run(x, toggle_gates, trace=False):
    if "nc" not in _CACHE:
        _CACHE["nc"] = _build()
    nc = _CACHE["nc"]

    x = np.asarray(x, dtype=np.float32)
    tg = np.asarray(toggle_gates, dtype=np.float32)
    tgT = np.ascontiguousarray(tg.reshape(L, NPOS, HW).transpose(0, 2, 1))
    in_maps = []
    for c in range(NCORES):
        xb = x[BLOC * c : BLOC * (c + 1)]
        in_maps.append({"g0": _host_g0(xb), "tgt": tgT})

    res = run_bass_kernel_spmd(nc, in_maps, core_ids=list(range(NCORES)), trace=trace)
    outs = []
    for c in range(NCORES):
        pm = np.asarray(res.results[c]["out"])  # [128, 16]
        for b in range(BLOC):
            outs.append(pm[:, b * 8 : (b + 1) * 8].T.reshape(32, 32))
    full = np.stack(outs, axis=0)
    return full, res


def kernel(x, toggle_gates):
    full, _ = _run(x, toggle_gates, trace=False)
    return full
